# revision 22
# baseline (speedup 1.0000x reference)
"""Trainium2 Bass kernel for nn_CLLayer (SimCLR-style contrastive loss).

Math (reference, tau=0.5):
    h1 = elu(z1 @ W1.T + b1) @ W2.T + b2 ; h2 likewise
    n1, n2 = row-normalized h1, h2
    l1_i = log(sum_j exp(2*n1_i.n1_j) + sum_j exp(2*n1_i.n2_j) - e^2) - 2*n1_i.n2_i
    l2_i = log(sum_j exp(2*n2_i.n2_j) + sum_j exp(2*n1_j.n2_i) - e^2) - 2*n1_i.n2_i
    out = 0.5*(l1+l2)

Strategy (row-parallel over N=8192, 1024 rows/core, 8 cores):
 - FP8(e4m3) DoubleRow matmuls everywhere (projection + similarity).
 - S11/S22 symmetric: cyclic per-core distribution at 512-row strip
   granularity using partition_id-driven dynamic DMA offsets. Core c's
   row strip r (r = 2c+h) computes cells (r, (r+e) mod 16) for e=0..8:
   e=0 is the local diagonal (rhs straight from SBUF, no AllGather
   dependency), e=1..7 full weight, e=8 at half weight (exp bias ln1/2;
   the partner strip computes the transposed cell also at half). Row
   sums are local (activation accum_out); the transposed halves arrive
   as per-strip column sums via ReduceScatter.
 - S12 not symmetric -> 8 full 1024-blocks, rotated (c+t)%8 so t=0 is
   the local block (SBUF rhs, no AllGather); its column sums feed l2.
 - exp tiles are fp8; [128,1024]-wide exp instructions wherever the
   bias is uniform across the two 512-col streams. Column sums are
   ones-vector DoubleRow matmuls accumulated in PSUM, deferred one
   group so the PE never waits on Act.
 - All DRAM<->SBUF layouts are [128, KO, x] (contiguous); host
   pre-arranges inputs. Normalized embeddings are scaled x16, cast fp8
   before the fp8 AllGather; sim psums are descaled in the exp
   (scale=2/256). pos diag p_i = n1_i.n2_i from a separate bf16 path.
 - Schedule: proj1(z1), proj1(z2), proj2(h1), norm(h1), AG1,
   proj2(h2), norm(h2), AG2, p-path + all-local sim cells (covers the
   AG1 mesh), S11 remote, S12 remote, RS(S11+S12 colsums) under S22
   remote, RS(S22 colsums), tail.

Host-side prep: K-major [ki, ko, x] arrangement, fp8 casts, and
b2' = b2 - sum_k W2_f8 so ELU is computed as relu(x) + min(exp(x),1)
(device ELU' = elu + 1).
"""

import math
import os
from functools import lru_cache

import ml_dtypes
import numpy as np

import concourse.bacc as bacc
import concourse.bass as bass
import concourse.mybir as mybir
import concourse.tile as tile
from concourse.bass_utils import run_bass_kernel_spmd

N, D = 8192, 1024
NCORES = 8
BLK = N // NCORES  # 1024
P = 128
KO = D // P  # 8 k-tiles
KO2 = KO // 2  # 4 double-row k-pairs
NT = BLK // P  # 8 i-tiles per core
E2 = float(np.exp(2.0))  # exp(1/tau), tau=0.5
SC = 2.0 / 256.0  # exp scale: tau and the 16x16 fp8 prescale
LN_HALF = float(math.log(0.5))
BF = mybir.dt.bfloat16
F32 = mybir.dt.float32
F8 = mybir.dt.float8e4
AF = mybir.ActivationFunctionType
ALU = mybir.AluOpType
DR = mybir.MatmulPerfMode.DoubleRow

# D1p slot layout: S11 uses 0..5 (h0: 5, h1: 6), S12 uses 6..13
S12_SLOT0 = 6
ND1 = 14
ND2 = 6


def _build():
    nc = bacc.Bacc("TRN2", target_bir_lowering=False, debug=False, num_devices=NCORES)

    # all tensors arrive pre-arranged as [ki=128, ko, x] (contiguous loads)
    z1t = nc.dram_tensor("z1t", [P, KO, BLK], F8, kind="ExternalInput")
    z2t = nc.dram_tensor("z2t", [P, KO, BLK], F8, kind="ExternalInput")
    w1t = nc.dram_tensor("w1t", [P, KO, D], F8, kind="ExternalInput")
    w2t = nc.dram_tensor("w2t", [P, KO, D], F8, kind="ExternalInput")
    b1 = nc.dram_tensor("b1", [D], F32, kind="ExternalInput")
    b2p = nc.dram_tensor("b2p", [D], F32, kind="ExternalInput")
    out = nc.dram_tensor("out", [BLK], F32, kind="ExternalOutput")

    pt = lambda ap: ap.rearrange("(t p) -> p t", p=P)  # [1024] -> [128, 8]

    with tile.TileContext(nc) as tc:
        with (
            tc.tile_pool(name="consts", bufs=1) as consts,
            tc.tile_pool(name="mats", bufs=1) as mats,
            tc.tile_pool(name="strip", bufs=1) as strip,
            tc.tile_pool(name="scratch", bufs=2) as scratch,
            tc.tile_pool(name="rhs", bufs=4) as rhsp,
            tc.tile_pool(name="expp", bufs=6) as expp,
            tc.tile_pool(name="small", bufs=1) as small,
            tc.tile_pool(name="psA", bufs=3, space="PSUM") as psA,
            tc.tile_pool(name="psB", bufs=2, space="PSUM") as psB,
            tc.tile_pool(name="dram", bufs=1, space="DRAM") as dram,
        ):
            pid_sp = nc.sync.partition_id()
            pid_gp = nc.gpsimd.partition_id()

            # ---------------- constants ----------------
            w1_sb = consts.tile([P, KO, D], F8)
            w2_sb = consts.tile([P, KO, D], F8)
            b1_sb = consts.tile([P, KO], F32)
            b2_sb = consts.tile([P, KO], F32)
            z1_sb = mats.tile([P, KO, BLK], F8, tag="z1")
            z2_sb = mats.tile([P, KO, BLK], F8, tag="z2")
            # k-chunked so the first matmuls can start after the first chunk
            for k2 in range(KO2):
                ksl = slice(2 * k2, 2 * k2 + 2)
                nc.sync.dma_start(w1_sb[:, ksl, :], w1t[:, ksl, :])
                nc.sync.dma_start(z1_sb[:, ksl, :], z1t[:, ksl, :])
            nc.sync.dma_start(w2_sb[:], w2t[:])
            nc.sync.dma_start(z2_sb[:], z2t[:])
            nc.sync.dma_start(b1_sb[:], pt(b1[:]))
            nc.sync.dma_start(b2_sb[:], pt(b2p[:]))
            ones8 = consts.tile([P, 2, 16], F8)
            ones_bf = consts.tile([P, 1], BF)
            lnhalf = consts.tile([P, 1], F32)
            negE2 = consts.tile([P, 1], F32)
            nc.vector.memset(ones8[:], 1.0)
            nc.vector.memset(ones_bf[:], 1.0)
            nc.vector.memset(lnhalf[:], LN_HALF)
            nc.vector.memset(negE2[:], -E2)

            h1_sb = mats.tile([P, KO, BLK], BF, tag="h1")
            h2_sb = mats.tile([P, KO, BLK], BF, tag="h2")
            n1_f8 = mats.tile([P, KO, BLK], F8, tag="n1")
            n2_f8 = mats.tile([P, KO, BLK], F8, tag="n2")

            ag1_in = dram.tile([P, KO, BLK], F8)
            ag2_in = dram.tile([P, KO, BLK], F8)
            ag1_out = dram.tile([NCORES, P, KO, BLK], F8, addr_space="Shared")
            ag2_out = dram.tile([NCORES, P, KO, BLK], F8, addr_space="Shared")
            rs_in_a = dram.tile([NCORES, 2, BLK], F32)  # S11 / S12 colsums
            rs_in_b = dram.tile([NCORES, BLK], F32)  # S22 colsums
            rs_out_a = dram.tile([2, BLK], F32)
            rs_out_b = dram.tile([BLK], F32)
            p_dram = dram.tile([BLK], F32)

            # rowsum partials: one column per exp-instruction group
            D1p = strip.tile([P, NT, ND1], F32)
            D2p = strip.tile([P, NT, ND2], F32)
            nc.vector.memset(D1p[:], 0.0)
            nc.vector.memset(D2p[:], 0.0)

            # zero rs buffers: each core only exports colsums for the strips
            # it computed; the ReduceScatter sums whole buffers
            zt = consts.tile([1, BLK], F32)
            nc.vector.memset(zt[:], 0.0)
            for j in range(NCORES):
                nc.gpsimd.dma_start(rs_in_a[j, 0], zt[:])
                nc.gpsimd.dma_start(rs_in_a[j, 1], zt[:])
                nc.gpsimd.dma_start(rs_in_b[j], zt[:])

            rn_f = [
                small.tile([1, BLK], F32, tag=f"rn_f{i}", name=f"rn_f{i}")
                for i in range(2)
            ]

            def dr_multi(ps_list, lhs, tt, rhs_list):
                """K=1024 fp8 DoubleRow accumulation over several (ps, rhs)
                streams sharing the same stationary lhs tile per k-pair."""
                for k2 in range(KO2):
                    lslice = lhs[:, 2 * k2 : 2 * k2 + 2, bass.ts(tt, P)]
                    for ps_sl, (rt, col) in zip(ps_list, rhs_list):
                        nc.tensor.matmul(
                            ps_sl,
                            lslice,
                            rt[:, 2 * k2 : 2 * k2 + 2, bass.ds(col, 512)],
                            start=(k2 == 0),
                            stop=(k2 == KO2 - 1),
                            perf_mode=DR,
                        )

            # ------------ projection + normalize, per tensor ------------
            def proj_layer(w_sb, src, emit_ot, ots=None):
                for ot in ots if ots is not None else range(KO):
                    ps = psA.tile([P, 1024], F32, tag="ps_big")
                    dr_multi(
                        [ps[:, 0:512], ps[:, 512:1024]],
                        w_sb, ot, [(src, 0), (src, 512)],
                    )
                    emit_ot(ot, ps)

            def proj_layer_rounds(w_sb, src, emit_ot):
                """k2-outer over 3-ot rounds: consumes src k-chunks at load
                pace, so the first layer starts before its DMA completes."""
                for rnd in ((0, 1, 2), (3, 4, 5), (6, 7)):
                    pss = [
                        psA.tile([P, 1024], F32, tag="ps_big", name=f"psr{ot}")
                        for ot in rnd
                    ]
                    for k2 in range(KO2):
                        for i, ot in enumerate(rnd):
                            lslice = w_sb[:, 2 * k2 : 2 * k2 + 2, bass.ts(ot, P)]
                            for col in (0, 512):
                                nc.tensor.matmul(
                                    pss[i][:, col : col + 512],
                                    lslice,
                                    src[:, 2 * k2 : 2 * k2 + 2, bass.ds(col, 512)],
                                    start=(k2 == 0),
                                    stop=(k2 == KO2 - 1),
                                    perf_mode=DR,
                                )
                    for i, ot in enumerate(rnd):
                        emit_ot(ot, pss[i])

            def l1_emit(elu_sb):
                def emit(ot, ps):
                    bcol = b1_sb[:, ot : ot + 1]
                    e_t = scratch.tile([P, 1024], BF, tag="e_t", bufs=3)
                    r_t = scratch.tile([P, 1024], BF, tag="r_t", bufs=3)
                    nc.scalar.activation(e_t[:], ps[:], AF.Exp, bias=bcol)
                    nc.scalar.activation(r_t[:], ps[:], AF.Relu, bias=bcol)
                    nc.vector.tensor_scalar(e_t[:], e_t[:], 1.0, None, ALU.min)
                    # final add alternates DVE / GpSimd to unclog the DVE queue
                    eng = nc.vector if ot % 2 == 0 else nc.gpsimd
                    eng.tensor_tensor(elu_sb[:, ot, :], e_t[:], r_t[:], ALU.add)
                return emit

            def l2_emit(h_sb):
                def emit(ot, ps):
                    if ot % 2 == 0:
                        nc.scalar.activation(
                            h_sb[:, ot, :], ps[:], AF.Identity, bias=b2_sb[:, ot : ot + 1]
                        )
                    else:
                        nc.vector.tensor_scalar(
                            h_sb[:, ot, :], ps[:], b2_sb[:, ot : ot + 1], None, ALU.add
                        )
                return emit

            def normalize(h_sb, n_f8, rn_slot):
                # sumsq over d via bf16 ones-matmul on h*h
                ssps = [
                    psB.tile([16, 512], F32, name=f"ssps{rn_slot}_{c}", tag="ps_wide")
                    for c in range(2)
                ]
                for kt in range(KO):
                    sq = scratch.tile([P, BLK], BF, tag="sq")
                    nc.vector.tensor_tensor(sq[:], h_sb[:, kt, :], h_sb[:, kt, :], ALU.mult)
                    for ch in range(2):
                        nc.tensor.matmul(
                            ssps[ch][0:1, :],
                            ones_bf[:],
                            sq[:, bass.ts(ch, 512)],
                            start=(kt == 0),
                            stop=(kt == KO - 1),
                        )
                # rn = 1/||h|| = sqrt(1/ssq): DVE fast reciprocal (18-bit, far
                # below the fp8 grain) + Act Sqrt; scale=256 folds in the x16
                # fp8 prescale. Short chain -> AG launches early.
                rn16_bf = small.tile([1, BLK], BF, tag="rn16_bf", name=f"rn16_{rn_slot}")
                ssq_c = small.tile([1, BLK], F32, tag="ssq_c", name=f"ssq{rn_slot}")
                y_c = small.tile([1, BLK], F32, tag="y_c", name=f"y{rn_slot}")
                for ch in range(2):
                    nc.vector.tensor_copy(ssq_c[:, bass.ts(ch, 512)], ssps[ch][0:1, :])
                nc.vector.reciprocal_approx_fast(y_c[:], ssq_c[:])
                nc.scalar.activation(rn16_bf[:], y_c[:], AF.Sqrt, scale=256.0)
                nc.scalar.activation(rn_f[rn_slot][:], y_c[:], AF.Sqrt)
                rn_bc = scratch.tile([P, BLK], BF, tag="rnbc", bufs=2, name=f"rnbc{rn_slot}")
                nc.gpsimd.partition_broadcast(rn_bc[:], rn16_bf[:])
                # split the scale+cast across DVE and GpSimd (AG-critical)
                for kt in range(KO):
                    eng = nc.vector if kt % 2 == 0 else nc.gpsimd
                    eng.tensor_tensor(
                        n_f8[:, kt, :], h_sb[:, kt, :], rn_bc[:], ALU.mult
                    )

            rg = [list(range(NCORES))]
            elu1 = mats.tile([P, KO, BLK], F8, tag="elu", name="elu1")
            elu2 = mats.tile([P, KO, BLK], F8, tag="z1", name="elu2")  # z1 dead post-l1

            # z1's full chain first so AG1 launches as early as possible;
            # the whole z2 chain then overlaps the AG1 mesh
            proj_layer_rounds(w1_sb, z1_sb, l1_emit(elu1))
            proj_layer(w2_sb, elu1, l2_emit(h1_sb))
            normalize(h1_sb, n1_f8, 0)
            nc.sync.dma_start(ag1_in[:], n1_f8[:])
            nc.gpsimd.collective_compute(
                "AllGather", ALU.bypass, replica_groups=rg,
                ins=[ag1_in[:].opt()], outs=[ag1_out[:].opt()],
            )
            proj_layer(w1_sb, z2_sb, l1_emit(elu2))
            proj_layer(w2_sb, elu2, l2_emit(h2_sb))
            normalize(h2_sb, n2_f8, 1)
            nc.sync.dma_start(ag2_in[:], n2_f8[:])
            nc.gpsimd.collective_compute(
                "AllGather", ALU.bypass, replica_groups=rg,
                ins=[ag2_in[:].opt()], outs=[ag2_out[:].opt()],
            )

            # ------ p_i = n1_i . n2_i via bf16 h1*h2 and f32 1/norms ------
            pps = [
                psB.tile([16, 512], F32, name=f"pps{c}", tag="ps_wide") for c in range(2)
            ]
            for kt in range(KO):
                hq = scratch.tile([P, BLK], BF, tag="sq", name=f"hq{kt}")
                nc.vector.tensor_tensor(hq[:], h1_sb[:, kt, :], h2_sb[:, kt, :], ALU.mult)
                for ch in range(2):
                    nc.tensor.matmul(
                        pps[ch][0:1, :],
                        ones_bf[:],
                        hq[:, bass.ts(ch, 512)],
                        start=(kt == 0),
                        stop=(kt == KO - 1),
                    )
            for ch in range(2):
                sl = bass.ts(ch, 512)
                p_c = small.tile([1, 512], F32, tag="p_c", bufs=2, name=f"p_c{ch}")
                nc.vector.tensor_copy(p_c[:], pps[ch][0:1, :])
                nc.vector.tensor_tensor(p_c[:], p_c[:], rn_f[0][:, sl], ALU.mult)
                nc.vector.tensor_tensor(p_c[:], p_c[:], rn_f[1][:, sl], ALU.mult)
                nc.gpsimd.dma_start(p_dram[ch * 512 : (ch + 1) * 512], p_c[:])

            # ---------------- similarity passes ----------------
            # colsums deferred to the next group so the PE never waits on
            # the Act engine's exp outputs
            pending = []

            def flush_pending():
                while pending:
                    pending.pop(0)()

            def colsum(jdyn, hh, rs_sel, src_aps, nm):
                """PSUM-accumulated fp8 ones DoubleRow colsum of [128,2,512]
                exp slices -> dynamic rs slot (block jdyn, 512-col half hh).
                rs_sel: 0/1 -> rs_in_a slot, 2 -> rs_in_b."""

                def emit():
                    cps = psB.tile([16, 512], F32, tag="ps_wide", name=f"cps{nm}")
                    for i, ap in enumerate(src_aps):
                        nc.tensor.matmul(
                            cps[:], ones8[:], ap,
                            start=(i == 0), stop=(i == len(src_aps) - 1),
                            perf_mode=DR,
                        )
                    cst = scratch.tile([1, 512], F32, tag="cst", bufs=2, name=f"cst{nm}")
                    nc.vector.tensor_copy(cst[:], cps[0:1, :])
                    if rs_sel == 2:
                        dst = rs_in_b[bass.ds(jdyn, 1), hh * 512 : hh * 512 + 512]
                    else:
                        dst = rs_in_a[
                            bass.ds(jdyn, 1), rs_sel : rs_sel + 1,
                            hh * 512 : hh * 512 + 512,
                        ]
                    nc.gpsimd.dma_start(dst, cst[:])

                pending.append(emit)

            def load_block(ag, m, nm):
                jj = (pid_sp + m) % NCORES
                t = rhsp.tile([P, KO, BLK], F8, tag="rhs", name=nm)
                nc.sync.dma_start(t[:, 0:4, :], ag[bass.ds(jj, 1), :, 0:4, :])
                nc.sync.dma_start(t[:, 4:8, :], ag[bass.ds(jj, 1), :, 4:8, :])
                return t

            def sym_group(n_f8, rt, m, Dp, rs_sel, nm):
                """One cyclic m-group of a symmetric matrix: cells of both
                local row halves h against block (c+m): strips 2c+2m, 2c+2m+1.

                m=0: local rhs (rt = n_f8): h0 pair (e0,e1), h1 lone diag e0.
                m=1..3: h0 pair (e2m,e2m+1), h1 pair (e2m-1,e2m), full wt.
                m=4: h0 lone e8 (half wt), h1 pair (e7, e8 at half wt).
                Dp accum slots: h0 -> m; h1 -> m (m<4), m4 -> 4,5."""
                first = True
                ex_half = {0: [], 1: []}  # strip half -> colsum source APs
                for h in (0, 1):
                    # (col, half-weight?, colsummed?) per stream
                    if m == 0:
                        streams = (
                            [(0, False, False), (512, False, True)]
                            if h == 0
                            else [(512, False, False)]
                        )
                    elif m < 4:
                        streams = [(0, False, True), (512, False, True)]
                    elif h == 0:
                        streams = [(0, True, True)]
                    else:
                        streams = [(0, False, True), (512, True, True)]
                    nst = len(streams)
                    wide = nst == 2 and streams[0][1] == streams[1][1]
                    if wide:
                        ext = [
                            expp.tile([P, 4, 1024], F8, tag="exw", bufs=4,
                                      name=f"xw{nm}{h}")
                        ]
                    else:
                        ext = [
                            expp.tile([P, 4, 512], F8, tag="exn", bufs=4,
                                      name=f"xn{nm}{h}{i}")
                            for i in range(nst)
                        ]
                    for tl in range(4):
                        tt = 4 * h + tl
                        ps = psA.tile([P, 1024], F32, tag="ps_big", name=f"p{nm}{h}{tl}")
                        dr_multi(
                            [ps[:, 512 * i : 512 * (i + 1)] for i in range(nst)],
                            n_f8, tt, [(rt, col) for col, _, _ in streams],
                        )
                        if first:
                            flush_pending()
                            first = False
                        if wide:
                            nc.scalar.activation(
                                ext[0][:, tl, :], ps[:], AF.Exp, scale=SC,
                                bias=(lnhalf[:] if streams[0][1] else 0.0),
                                accum_out=Dp[:, tt, m : m + 1],
                            )
                        else:
                            for i, (col, halfw, _) in enumerate(streams):
                                sl = (4 + i) if (m == 4 and h == 1) else m
                                nc.scalar.activation(
                                    ext[i][:, tl, :],
                                    ps[:, 512 * i : 512 * (i + 1)],
                                    AF.Exp, scale=SC,
                                    bias=(lnhalf[:] if halfw else 0.0),
                                    accum_out=Dp[:, tt, sl : sl + 1],
                                )
                    # colsum sources per strip half of block (c+m):
                    # stream i covers strip half i, except m0 (h0 stream1 ->
                    # half 1) and m4-h0 lone (e8 -> half 0).
                    for i, (col, halfw, summed) in enumerate(streams):
                        if not summed:
                            continue
                        hh = i
                        if m == 0:
                            hh = 1
                        elif m == 4 and h == 0:
                            hh = 0
                        for tp in range(2):
                            if wide:
                                ex_half[hh].append(
                                    ext[0][:, 2 * tp : 2 * tp + 2,
                                           512 * i : 512 * i + 512]
                                )
                            else:
                                ex_half[hh].append(ext[i][:, 2 * tp : 2 * tp + 2, :])
                jgp = (pid_gp + m) % NCORES
                for hh in (0, 1):
                    if ex_half[hh]:
                        colsum(jgp, hh, rs_sel, ex_half[hh], f"{nm}h{hh}")

            def s12_block(t, nm):
                if t == 0:
                    rt = n2_f8
                else:
                    rt = load_block(ag2_out, t, f"r{nm}")
                exf = [
                    expp.tile([P, 2, 1024], F8, tag="exf", bufs=6, name=f"xf{nm}_{i}")
                    for i in range(4)
                ]
                for tt in range(NT):
                    ps = psA.tile([P, 1024], F32, tag="ps_big", name=f"pf{nm}_{tt}")
                    dr_multi(
                        [ps[:, 0:512], ps[:, 512:1024]],
                        n1_f8, tt, [(rt, 0), (rt, 512)],
                    )
                    if tt == 0:
                        flush_pending()
                    nc.scalar.activation(
                        exf[tt // 2][:, tt % 2, :], ps[:], AF.Exp, scale=SC,
                        accum_out=D1p[:, tt, S12_SLOT0 + t : S12_SLOT0 + t + 1],
                    )
                jgp = (pid_gp + t) % NCORES
                for hh in range(2):
                    colsum(
                        jgp, hh, 1,
                        [e[:, :, hh * 512 : hh * 512 + 512] for e in exf],
                        f"f{nm}_{hh}",
                    )

            # --- all-local sim work first: covers the AG1/AG2 meshes ---
            sym_group(n1_f8, n1_f8, 0, D1p, 0, "s11_0")
            s12_block(0, "t0")
            sym_group(n2_f8, n2_f8, 0, D2p, 2, "s22_0")
            # --- S11 remote (needs AG1) ---
            for m in range(1, 5):
                rt = load_block(ag1_out, m, f"rs11_{m}")
                sym_group(n1_f8, rt, m, D1p, 0, f"s11_{m}")
            # --- S12 remote (needs AG2) ---
            for t in range(1, NCORES):
                s12_block(t, f"t{t}")
            # --- S22 remote; RS_a + l1 partial under it ---
            for m in range(1, 5):
                rt = load_block(ag2_out, m, f"rs22_{m}")
                sym_group(n2_f8, rt, m, D2p, 2, f"s22_{m}")
                if m == 1:
                    # S11+S12 colsums all flushed -> reduce them under S22
                    nc.gpsimd.collective_compute(
                        "ReduceScatter", ALU.add, replica_groups=rg,
                        ins=[rs_in_a[:].opt()], outs=[rs_out_a[:].opt()],
                    )
                    # D1 side of the loss, precomputed while S22 runs
                    p2 = small.tile([P, NT], F32, tag="p2")
                    nc.sync.dma_start(p2[:], pt(p_dram[:]))
                    pm = small.tile([P, NT], F32, tag="pm")
                    nc.vector.tensor_scalar(pm[:], p2[:], -2.0, None, ALU.mult)
                    r1s = small.tile([P, NT], F32, tag="r1s")
                    nc.vector.reduce_sum(r1s[:], D1p[:], axis=mybir.AxisListType.X)
                    c1 = small.tile([P, NT], F32, tag="c1")
                    c2b = small.tile([P, NT], F32, tag="c2b")
                    nc.sync.dma_start(c1[:], pt(rs_out_a[0]))
                    nc.sync.dma_start(c2b[:], pt(rs_out_a[1]))
                    d1 = small.tile([P, NT], F32, tag="d1")
                    nc.vector.tensor_tensor(d1[:], r1s[:], c1[:], ALU.add)
                    l1 = small.tile([P, NT], F32, tag="l1")
                    nc.scalar.activation(l1[:], d1[:], AF.Ln, bias=negE2[:])
                    loss_a = small.tile([P, NT], F32, tag="loss_a")
                    nc.vector.tensor_scalar(loss_a[:], l1[:], 0.5, None, ALU.mult)
                    nc.vector.tensor_tensor(loss_a[:], loss_a[:], pm[:], ALU.add)
            flush_pending()
            # d2 partial (S22 rowsums + S12-colsum term) in parallel with RS_b
            r2s = small.tile([P, NT], F32, tag="r2s")
            nc.vector.reduce_sum(r2s[:], D2p[:], axis=mybir.AxisListType.X)
            d2 = small.tile([P, NT], F32, tag="d2")
            nc.vector.tensor_tensor(d2[:], r2s[:], c2b[:], ALU.add)

            nc.gpsimd.collective_compute(
                "ReduceScatter", ALU.add, replica_groups=rg,
                ins=[rs_in_b[:].opt()], outs=[rs_out_b[:].opt()],
            )

            # ---------------- final loss (tail) ----------------
            c2a = small.tile([P, NT], F32, tag="c2a")
            nc.sync.dma_start(c2a[:], pt(rs_out_b[:]))
            nc.vector.tensor_tensor(d2[:], d2[:], c2a[:], ALU.add)
            l2 = small.tile([P, NT], F32, tag="l2")
            nc.scalar.activation(l2[:], d2[:], AF.Ln, bias=negE2[:])
            loss = small.tile([P, NT], F32, tag="loss")
            nc.vector.tensor_scalar(loss[:], l2[:], 0.5, None, ALU.mult)
            nc.vector.tensor_tensor(loss[:], loss[:], loss_a[:], ALU.add)
            nc.sync.dma_start(pt(out[:]), loss[:])

    nc.finalize()
    return nc


@lru_cache(maxsize=1)
def _built():
    return _build()


def _kimajor(a):
    """[D, X] -> [ki=128, ko=8, X] contiguous device layout."""
    d, x = a.shape
    return np.ascontiguousarray(a.reshape(KO, P, x).transpose(1, 0, 2))


def _prep_inputs(z1, z2, fc1_w, fc1_b, fc2_w, fc2_b):
    f8 = ml_dtypes.float8_e4m3
    w1 = np.asarray(fc1_w, np.float32)
    w2 = np.asarray(fc2_w, np.float32)
    w1t = _kimajor(np.ascontiguousarray(w1.T).astype(f8))
    w2t = _kimajor(np.ascontiguousarray(w2.T).astype(f8))
    b1 = np.asarray(fc1_b, np.float32)
    # device computes (elu+1) @ W2.T; correct with the fp8-rounded W2 row sums
    b2p = (
        np.asarray(fc2_b, np.float32) - w2.astype(f8).astype(np.float32).sum(axis=1)
    ).astype(np.float32)
    in_maps = []
    for c in range(NCORES):
        sl = slice(c * BLK, (c + 1) * BLK)
        in_maps.append(
            {
                "z1t": _kimajor(np.asarray(z1[sl], np.float32).T.astype(f8)),
                "z2t": _kimajor(np.asarray(z2[sl], np.float32).T.astype(f8)),
                "w1t": w1t,
                "w2t": w2t,
                "b1": b1,
                "b2p": b2p,
            }
        )
    return in_maps


def _install_ntff_shim():
    """Register the axon NTFF profile hook (antenv.axon_hooks is absent in
    this image; rebuild it from trn_agent_boot's ctypes recipe)."""
    import sys
    import types

    if "antenv.axon_hooks" in sys.modules:
        return True
    try:
        import antenv
        from trn_agent_boot.trn_boot import _ntff_profile_via_ctypes

        hook = _ntff_profile_via_ctypes("/opt/axon/libaxon_pjrt.so")
        if hook is None:
            return False
        m = types.ModuleType("antenv.axon_hooks")
        m._hook = hook
        m.get_axon_ntff_profile_hook = lambda: m._hook
        m.set_axon_ntff_profile_hook = lambda h: setattr(m, "_hook", h)
        sys.modules["antenv.axon_hooks"] = m
        antenv.axon_hooks = m
        # artifact upload needs egress; neuter it for local profiling
        import concourse.bass_utils as _bu

        _bu.upload_artifacts = lambda tmpdir: f"file://{tmpdir}"
        return True
    except Exception as e:
        print(f"ntff shim unavailable: {e!r}")
        return False


def _run(in_maps, trace=False):
    nc = _built()
    if trace and not _install_ntff_shim():
        trace = False
    last = None
    for attempt in range(3):
        try:
            res = run_bass_kernel_spmd(nc, in_maps, list(range(NCORES)), trace=trace)
            if all(np.isfinite(res.results[c]["out"]).all() for c in range(NCORES)):
                return res
            print("nonfinite output, retrying")
        except Exception as e:  # device occasionally wedged from a prior process
            last = e
            if "UNRECOVERABLE" not in str(e) and "UNAVAILABLE" not in str(e):
                raise
            print(f"device error (attempt {attempt}): retrying")
    if last is not None:
        raise last
    return res


def kernel(z1, z2, fc1_w, fc1_b, fc2_w, fc2_b):
    in_maps = _prep_inputs(z1, z2, fc1_w, fc1_b, fc2_w, fc2_b)
    res = _run(in_maps, trace=os.environ.get("KERNEL_TRACE", "") == "1")
    if res.exec_time_ns is not None:
        print(f"HW exec time: {res.exec_time_ns} ns")
    out = np.concatenate([res.results[c]["out"] for c in range(NCORES)])
    return out.astype(np.float32)


# revision 23
# speedup vs baseline: 1.0720x; 1.0720x over previous
"""Trainium2 Bass kernel for nn_CLLayer (SimCLR-style contrastive loss).

Math (reference, tau=0.5):
    h1 = elu(z1 @ W1.T + b1) @ W2.T + b2 ; h2 likewise
    n1, n2 = row-normalized h1, h2
    l1_i = log(sum_j exp(2*n1_i.n1_j) + sum_j exp(2*n1_i.n2_j) - e^2) - 2*n1_i.n2_i
    l2_i = log(sum_j exp(2*n2_i.n2_j) + sum_j exp(2*n1_j.n2_i) - e^2) - 2*n1_i.n2_i
    out = 0.5*(l1+l2)

Strategy (row-parallel over N=8192, 1024 rows/core, 8 cores):
 - FP8(e4m3) DoubleRow matmuls everywhere (projection + similarity).
 - S11/S22 symmetric: cyclic per-core distribution at 512-row strip
   granularity using partition_id-driven dynamic DMA offsets. Core c's
   row strip r (r = 2c+h) computes cells (r, (r+e) mod 16) for e=0..8:
   e=0 is the local diagonal (rhs straight from SBUF, no AllGather
   dependency), e=1..7 full weight, e=8 at half weight (exp bias ln1/2;
   the partner strip computes the transposed cell also at half). Row
   sums are local (activation accum_out); the transposed halves arrive
   as per-strip column sums via ReduceScatter.
 - S12 not symmetric -> 8 full 1024-blocks, rotated (c+t)%8 so t=0 is
   the local block (SBUF rhs, no AllGather); its column sums feed l2.
 - exp tiles are fp8; [128,1024]-wide exp instructions wherever the
   bias is uniform across the two 512-col streams. Column sums are
   ones-vector DoubleRow matmuls accumulated in PSUM, deferred one
   group so the PE never waits on Act.
 - All DRAM<->SBUF layouts are [128, KO, x] (contiguous); host
   pre-arranges inputs. Normalized embeddings are scaled x16, cast fp8
   before the fp8 AllGather; sim psums are descaled in the exp
   (scale=2/256). pos diag p_i = n1_i.n2_i from a separate bf16 path.
 - Schedule: proj1(z1), proj1(z2), proj2(h1), norm(h1), AG1,
   proj2(h2), norm(h2), AG2, p-path + all-local sim cells (covers the
   AG1 mesh), S11 remote, S12 remote, RS(S11+S12 colsums) under S22
   remote, RS(S22 colsums), tail.

Host-side prep: K-major [ki, ko, x] arrangement, fp8 casts, and
b2' = b2 - sum_k W2_f8 so ELU is computed as relu(x) + min(exp(x),1)
(device ELU' = elu + 1).
"""

import math
import os
from functools import lru_cache

import ml_dtypes
import numpy as np

import concourse.bacc as bacc
import concourse.bass as bass
import concourse.mybir as mybir
import concourse.tile as tile
from concourse.bass_utils import run_bass_kernel_spmd

N, D = 8192, 1024
NCORES = 8
BLK = N // NCORES  # 1024
P = 128
KO = D // P  # 8 k-tiles
KO2 = KO // 2  # 4 double-row k-pairs
NT = BLK // P  # 8 i-tiles per core
E2 = float(np.exp(2.0))  # exp(1/tau), tau=0.5
SC = 2.0 / 256.0  # exp scale: tau and the 16x16 fp8 prescale
LN_HALF = float(math.log(0.5))
BF = mybir.dt.bfloat16
F32 = mybir.dt.float32
F8 = mybir.dt.float8e4
AF = mybir.ActivationFunctionType
ALU = mybir.AluOpType
DR = mybir.MatmulPerfMode.DoubleRow

# D1p slot layout: S11 uses 0..5 (h0: 5, h1: 6), S12 uses 6..13
S12_SLOT0 = 6
ND1 = 14
ND2 = 6


def _build():
    nc = bacc.Bacc("TRN2", target_bir_lowering=False, debug=False, num_devices=NCORES)

    # all tensors arrive pre-arranged as [ki=128, ko, x] (contiguous loads)
    z1t = nc.dram_tensor("z1t", [P, KO, BLK], F8, kind="ExternalInput")
    z2t = nc.dram_tensor("z2t", [P, KO, BLK], F8, kind="ExternalInput")
    w1t = nc.dram_tensor("w1t", [P, KO, D], F8, kind="ExternalInput")
    w2t = nc.dram_tensor("w2t", [P, KO, D], F8, kind="ExternalInput")
    b1 = nc.dram_tensor("b1", [D], F32, kind="ExternalInput")
    b2p = nc.dram_tensor("b2p", [D], F32, kind="ExternalInput")
    out = nc.dram_tensor("out", [BLK], F32, kind="ExternalOutput")

    pt = lambda ap: ap.rearrange("(t p) -> p t", p=P)  # [1024] -> [128, 8]

    with tile.TileContext(nc) as tc:
        with (
            tc.tile_pool(name="consts", bufs=1) as consts,
            tc.tile_pool(name="mats", bufs=1) as mats,
            tc.tile_pool(name="strip", bufs=1) as strip,
            tc.tile_pool(name="scratch", bufs=2) as scratch,
            tc.tile_pool(name="rhs", bufs=4) as rhsp,
            tc.tile_pool(name="expp", bufs=6) as expp,
            tc.tile_pool(name="small", bufs=1) as small,
            tc.tile_pool(name="psA", bufs=3, space="PSUM") as psA,
            tc.tile_pool(name="psB", bufs=2, space="PSUM") as psB,
            tc.tile_pool(name="dram", bufs=1, space="DRAM") as dram,
        ):
            pid_sp = nc.sync.partition_id()
            pid_gp = nc.gpsimd.partition_id()

            # ---------------- constants ----------------
            w1_sb = consts.tile([P, KO, D], F8)
            w2_sb = consts.tile([P, KO, D], F8)
            b1_sb = consts.tile([P, KO], F32)
            b2_sb = consts.tile([P, KO], F32)
            z1_sb = mats.tile([P, KO, BLK], F8, tag="z1")
            z2_sb = mats.tile([P, KO, BLK], F8, tag="z2")
            nc.sync.dma_start(w1_sb[:], w1t[:])
            nc.sync.dma_start(z1_sb[:], z1t[:])
            nc.sync.dma_start(w2_sb[:], w2t[:])
            nc.sync.dma_start(z2_sb[:], z2t[:])
            nc.sync.dma_start(b1_sb[:], pt(b1[:]))
            nc.sync.dma_start(b2_sb[:], pt(b2p[:]))
            ones8 = consts.tile([P, 2, 16], F8)
            ones_bf = consts.tile([P, 1], BF)
            lnhalf = consts.tile([P, 1], F32)
            negE2 = consts.tile([P, 1], F32)
            nc.vector.memset(ones8[:], 1.0)
            nc.vector.memset(ones_bf[:], 1.0)
            nc.vector.memset(lnhalf[:], LN_HALF)
            nc.vector.memset(negE2[:], -E2)

            h1_sb = mats.tile([P, KO, BLK], BF, tag="h1")
            h2_sb = mats.tile([P, KO, BLK], BF, tag="h2")
            n1_f8 = mats.tile([P, KO, BLK], F8, tag="n1")
            n2_f8 = mats.tile([P, KO, BLK], F8, tag="n2")

            ag1_in = dram.tile([P, KO, BLK], F8)
            ag2_in = dram.tile([P, KO, BLK], F8)
            ag1_out = dram.tile([NCORES, P, KO, BLK], F8, addr_space="Shared")
            ag2_out = dram.tile([NCORES, P, KO, BLK], F8, addr_space="Shared")
            rs_in_a = dram.tile([NCORES, 2, BLK], F32)  # S11 / S12 colsums
            rs_in_b = dram.tile([NCORES, BLK], F32)  # S22 colsums
            rs_out_a = dram.tile([2, BLK], F32)
            rs_out_b = dram.tile([BLK], F32)
            p_dram = dram.tile([BLK], F32)

            # rowsum partials: one column per exp-instruction group
            D1p = strip.tile([P, NT, ND1], F32)
            D2p = strip.tile([P, NT, ND2], F32)
            nc.vector.memset(D1p[:], 0.0)
            nc.vector.memset(D2p[:], 0.0)

            # zero rs buffers: each core only exports colsums for the strips
            # it computed; the ReduceScatter sums whole buffers
            zt = consts.tile([1, BLK], F32)
            nc.vector.memset(zt[:], 0.0)
            for j in range(NCORES):
                nc.gpsimd.dma_start(rs_in_a[j, 0], zt[:])
                nc.gpsimd.dma_start(rs_in_a[j, 1], zt[:])
                nc.gpsimd.dma_start(rs_in_b[j], zt[:])

            rn_f = [
                small.tile([1, BLK], F32, tag=f"rn_f{i}", name=f"rn_f{i}")
                for i in range(2)
            ]

            def dr_multi(ps_list, lhs, tt, rhs_list):
                """K=1024 fp8 DoubleRow accumulation over several (ps, rhs)
                streams sharing the same stationary lhs tile per k-pair."""
                for k2 in range(KO2):
                    lslice = lhs[:, 2 * k2 : 2 * k2 + 2, bass.ts(tt, P)]
                    for ps_sl, (rt, col) in zip(ps_list, rhs_list):
                        nc.tensor.matmul(
                            ps_sl,
                            lslice,
                            rt[:, 2 * k2 : 2 * k2 + 2, bass.ds(col, 512)],
                            start=(k2 == 0),
                            stop=(k2 == KO2 - 1),
                            perf_mode=DR,
                        )

            # ------------ projection + normalize, per tensor ------------
            def proj_layer(w_sb, src, emit_ot, ots=None):
                for ot in ots if ots is not None else range(KO):
                    ps = psA.tile([P, 1024], F32, tag="ps_big")
                    dr_multi(
                        [ps[:, 0:512], ps[:, 512:1024]],
                        w_sb, ot, [(src, 0), (src, 512)],
                    )
                    emit_ot(ot, ps)

            def proj_layer_rounds(w_sb, src, emit_ot):
                """k2-outer over 3-ot rounds: consumes src k-chunks at load
                pace, so the first layer starts before its DMA completes."""
                for rnd in ((0, 1, 2), (3, 4, 5), (6, 7)):
                    pss = [
                        psA.tile([P, 1024], F32, tag="ps_big", name=f"psr{ot}")
                        for ot in rnd
                    ]
                    for k2 in range(KO2):
                        for i, ot in enumerate(rnd):
                            lslice = w_sb[:, 2 * k2 : 2 * k2 + 2, bass.ts(ot, P)]
                            for col in (0, 512):
                                nc.tensor.matmul(
                                    pss[i][:, col : col + 512],
                                    lslice,
                                    src[:, 2 * k2 : 2 * k2 + 2, bass.ds(col, 512)],
                                    start=(k2 == 0),
                                    stop=(k2 == KO2 - 1),
                                    perf_mode=DR,
                                )
                    for i, ot in enumerate(rnd):
                        emit_ot(ot, pss[i])

            def l1_emit(elu_sb):
                def emit(ot, ps):
                    bcol = b1_sb[:, ot : ot + 1]
                    e_t = scratch.tile([P, 1024], BF, tag="e_t", bufs=3)
                    r_t = scratch.tile([P, 1024], BF, tag="r_t", bufs=3)
                    nc.scalar.activation(e_t[:], ps[:], AF.Exp, bias=bcol)
                    nc.scalar.activation(r_t[:], ps[:], AF.Relu, bias=bcol)
                    nc.vector.tensor_scalar(e_t[:], e_t[:], 1.0, None, ALU.min)
                    nc.vector.tensor_tensor(elu_sb[:, ot, :], e_t[:], r_t[:], ALU.add)
                return emit

            def l2_emit(h_sb):
                def emit(ot, ps):
                    if ot % 2 == 0:
                        nc.scalar.activation(
                            h_sb[:, ot, :], ps[:], AF.Identity, bias=b2_sb[:, ot : ot + 1]
                        )
                    else:
                        nc.vector.tensor_scalar(
                            h_sb[:, ot, :], ps[:], b2_sb[:, ot : ot + 1], None, ALU.add
                        )
                return emit

            def normalize(h_sb, n_f8, rn_slot):
                # sumsq over d via bf16 ones-matmul on h*h
                ssps = [
                    psB.tile([16, 512], F32, name=f"ssps{rn_slot}_{c}", tag="ps_wide")
                    for c in range(2)
                ]
                for kt in range(KO):
                    sq = scratch.tile([P, BLK], BF, tag="sq")
                    nc.vector.tensor_tensor(sq[:], h_sb[:, kt, :], h_sb[:, kt, :], ALU.mult)
                    for ch in range(2):
                        nc.tensor.matmul(
                            ssps[ch][0:1, :],
                            ones_bf[:],
                            sq[:, bass.ts(ch, 512)],
                            start=(kt == 0),
                            stop=(kt == KO - 1),
                        )
                # rn = 1/||h|| = sqrt(1/ssq): DVE fast reciprocal (18-bit, far
                # below the fp8 grain) + Act Sqrt; scale=256 folds in the x16
                # fp8 prescale. Short chain -> AG launches early.
                rn16_bf = small.tile([1, BLK], BF, tag="rn16_bf", name=f"rn16_{rn_slot}")
                ssq_c = small.tile([1, BLK], F32, tag="ssq_c", name=f"ssq{rn_slot}")
                y_c = small.tile([1, BLK], F32, tag="y_c", name=f"y{rn_slot}")
                for ch in range(2):
                    nc.vector.tensor_copy(ssq_c[:, bass.ts(ch, 512)], ssps[ch][0:1, :])
                nc.vector.reciprocal_approx_fast(y_c[:], ssq_c[:])
                nc.scalar.activation(rn16_bf[:], y_c[:], AF.Sqrt, scale=256.0)
                nc.scalar.activation(rn_f[rn_slot][:], y_c[:], AF.Sqrt)
                rn_bc = scratch.tile([P, BLK], BF, tag="rnbc", bufs=2, name=f"rnbc{rn_slot}")
                nc.gpsimd.partition_broadcast(rn_bc[:], rn16_bf[:])
                for kt in range(KO):
                    nc.vector.tensor_tensor(
                        n_f8[:, kt, :], h_sb[:, kt, :], rn_bc[:], ALU.mult
                    )

            rg = [list(range(NCORES))]
            elu1 = mats.tile([P, KO, BLK], F8, tag="elu", name="elu1")
            elu2 = mats.tile([P, KO, BLK], F8, tag="z1", name="elu2")  # z1 dead post-l1

            # z1's full chain first so AG1 launches as early as possible;
            # the whole z2 chain then overlaps the AG1 mesh
            proj_layer(w1_sb, z1_sb, l1_emit(elu1))
            proj_layer(w2_sb, elu1, l2_emit(h1_sb))
            normalize(h1_sb, n1_f8, 0)
            nc.sync.dma_start(ag1_in[:], n1_f8[:])
            nc.gpsimd.collective_compute(
                "AllGather", ALU.bypass, replica_groups=rg,
                ins=[ag1_in[:].opt()], outs=[ag1_out[:].opt()],
            )
            proj_layer(w1_sb, z2_sb, l1_emit(elu2))
            proj_layer(w2_sb, elu2, l2_emit(h2_sb))
            normalize(h2_sb, n2_f8, 1)
            nc.sync.dma_start(ag2_in[:], n2_f8[:])
            nc.gpsimd.collective_compute(
                "AllGather", ALU.bypass, replica_groups=rg,
                ins=[ag2_in[:].opt()], outs=[ag2_out[:].opt()],
            )

            # ------ p_i = n1_i . n2_i via bf16 h1*h2 and f32 1/norms ------
            pps = [
                psB.tile([16, 512], F32, name=f"pps{c}", tag="ps_wide") for c in range(2)
            ]
            for kt in range(KO):
                hq = scratch.tile([P, BLK], BF, tag="sq", name=f"hq{kt}")
                nc.vector.tensor_tensor(hq[:], h1_sb[:, kt, :], h2_sb[:, kt, :], ALU.mult)
                for ch in range(2):
                    nc.tensor.matmul(
                        pps[ch][0:1, :],
                        ones_bf[:],
                        hq[:, bass.ts(ch, 512)],
                        start=(kt == 0),
                        stop=(kt == KO - 1),
                    )
            for ch in range(2):
                sl = bass.ts(ch, 512)
                p_c = small.tile([1, 512], F32, tag="p_c", bufs=2, name=f"p_c{ch}")
                nc.vector.tensor_copy(p_c[:], pps[ch][0:1, :])
                nc.vector.tensor_tensor(p_c[:], p_c[:], rn_f[0][:, sl], ALU.mult)
                nc.vector.tensor_tensor(p_c[:], p_c[:], rn_f[1][:, sl], ALU.mult)
                nc.gpsimd.dma_start(p_dram[ch * 512 : (ch + 1) * 512], p_c[:])

            # ---------------- similarity passes ----------------
            # colsums deferred to the next group so the PE never waits on
            # the Act engine's exp outputs
            pending = []

            def flush_pending():
                while pending:
                    pending.pop(0)()

            def colsum(jdyn, hh, rs_sel, src_aps, nm):
                """PSUM-accumulated fp8 ones DoubleRow colsum of [128,2,512]
                exp slices -> dynamic rs slot (block jdyn, 512-col half hh).
                rs_sel: 0/1 -> rs_in_a slot, 2 -> rs_in_b."""

                def emit():
                    cps = psB.tile([16, 512], F32, tag="ps_wide", name=f"cps{nm}")
                    for i, ap in enumerate(src_aps):
                        nc.tensor.matmul(
                            cps[:], ones8[:], ap,
                            start=(i == 0), stop=(i == len(src_aps) - 1),
                            perf_mode=DR,
                        )
                    cst = scratch.tile([1, 512], F32, tag="cst", bufs=2, name=f"cst{nm}")
                    nc.vector.tensor_copy(cst[:], cps[0:1, :])
                    if rs_sel == 2:
                        dst = rs_in_b[bass.ds(jdyn, 1), hh * 512 : hh * 512 + 512]
                    else:
                        dst = rs_in_a[
                            bass.ds(jdyn, 1), rs_sel : rs_sel + 1,
                            hh * 512 : hh * 512 + 512,
                        ]
                    nc.gpsimd.dma_start(dst, cst[:])

                pending.append(emit)

            def load_block(ag, m, nm):
                jj = (pid_sp + m) % NCORES
                t = rhsp.tile([P, KO, BLK], F8, tag="rhs", name=nm)
                nc.sync.dma_start(t[:, 0:4, :], ag[bass.ds(jj, 1), :, 0:4, :])
                nc.sync.dma_start(t[:, 4:8, :], ag[bass.ds(jj, 1), :, 4:8, :])
                return t

            def sym_group(n_f8, rt, m, Dp, rs_sel, nm):
                """One cyclic m-group of a symmetric matrix: cells of both
                local row halves h against block (c+m): strips 2c+2m, 2c+2m+1.

                m=0: local rhs (rt = n_f8): h0 pair (e0,e1), h1 lone diag e0.
                m=1..3: h0 pair (e2m,e2m+1), h1 pair (e2m-1,e2m), full wt.
                m=4: h0 lone e8 (half wt), h1 pair (e7, e8 at half wt).
                Dp accum slots: h0 -> m; h1 -> m (m<4), m4 -> 4,5."""
                first = True
                ex_half = {0: [], 1: []}  # strip half -> colsum source APs
                for h in (0, 1):
                    # (col, half-weight?, colsummed?) per stream
                    if m == 0:
                        streams = (
                            [(0, False, False), (512, False, True)]
                            if h == 0
                            else [(512, False, False)]
                        )
                    elif m < 4:
                        streams = [(0, False, True), (512, False, True)]
                    elif h == 0:
                        streams = [(0, True, True)]
                    else:
                        streams = [(0, False, True), (512, True, True)]
                    nst = len(streams)
                    wide = nst == 2 and streams[0][1] == streams[1][1]
                    if wide:
                        ext = [
                            expp.tile([P, 4, 1024], F8, tag="exw", bufs=4,
                                      name=f"xw{nm}{h}")
                        ]
                    else:
                        ext = [
                            expp.tile([P, 4, 512], F8, tag="exn", bufs=4,
                                      name=f"xn{nm}{h}{i}")
                            for i in range(nst)
                        ]
                    for tl in range(4):
                        tt = 4 * h + tl
                        ps = psA.tile([P, 1024], F32, tag="ps_big", name=f"p{nm}{h}{tl}")
                        dr_multi(
                            [ps[:, 512 * i : 512 * (i + 1)] for i in range(nst)],
                            n_f8, tt, [(rt, col) for col, _, _ in streams],
                        )
                        if first:
                            flush_pending()
                            first = False
                        if wide:
                            nc.scalar.activation(
                                ext[0][:, tl, :], ps[:], AF.Exp, scale=SC,
                                bias=(lnhalf[:] if streams[0][1] else 0.0),
                                accum_out=Dp[:, tt, m : m + 1],
                            )
                        else:
                            for i, (col, halfw, _) in enumerate(streams):
                                sl = (4 + i) if (m == 4 and h == 1) else m
                                nc.scalar.activation(
                                    ext[i][:, tl, :],
                                    ps[:, 512 * i : 512 * (i + 1)],
                                    AF.Exp, scale=SC,
                                    bias=(lnhalf[:] if halfw else 0.0),
                                    accum_out=Dp[:, tt, sl : sl + 1],
                                )
                    # colsum sources per strip half of block (c+m):
                    # stream i covers strip half i, except m0 (h0 stream1 ->
                    # half 1) and m4-h0 lone (e8 -> half 0).
                    for i, (col, halfw, summed) in enumerate(streams):
                        if not summed:
                            continue
                        hh = i
                        if m == 0:
                            hh = 1
                        elif m == 4 and h == 0:
                            hh = 0
                        for tp in range(2):
                            if wide:
                                ex_half[hh].append(
                                    ext[0][:, 2 * tp : 2 * tp + 2,
                                           512 * i : 512 * i + 512]
                                )
                            else:
                                ex_half[hh].append(ext[i][:, 2 * tp : 2 * tp + 2, :])
                jgp = (pid_gp + m) % NCORES
                for hh in (0, 1):
                    if ex_half[hh]:
                        colsum(jgp, hh, rs_sel, ex_half[hh], f"{nm}h{hh}")

            def s12_block(t, nm):
                if t == 0:
                    rt = n2_f8
                else:
                    rt = load_block(ag2_out, t, f"r{nm}")
                exf = [
                    expp.tile([P, 2, 1024], F8, tag="exf", bufs=6, name=f"xf{nm}_{i}")
                    for i in range(4)
                ]
                for tt in range(NT):
                    ps = psA.tile([P, 1024], F32, tag="ps_big", name=f"pf{nm}_{tt}")
                    dr_multi(
                        [ps[:, 0:512], ps[:, 512:1024]],
                        n1_f8, tt, [(rt, 0), (rt, 512)],
                    )
                    if tt == 0:
                        flush_pending()
                    nc.scalar.activation(
                        exf[tt // 2][:, tt % 2, :], ps[:], AF.Exp, scale=SC,
                        accum_out=D1p[:, tt, S12_SLOT0 + t : S12_SLOT0 + t + 1],
                    )
                jgp = (pid_gp + t) % NCORES
                for hh in range(2):
                    colsum(
                        jgp, hh, 1,
                        [e[:, :, hh * 512 : hh * 512 + 512] for e in exf],
                        f"f{nm}_{hh}",
                    )

            # --- all-local sim work first: covers the AG1/AG2 meshes ---
            sym_group(n1_f8, n1_f8, 0, D1p, 0, "s11_0")
            s12_block(0, "t0")
            sym_group(n2_f8, n2_f8, 0, D2p, 2, "s22_0")
            # --- S11 remote (needs AG1) ---
            for m in range(1, 5):
                rt = load_block(ag1_out, m, f"rs11_{m}")
                sym_group(n1_f8, rt, m, D1p, 0, f"s11_{m}")
            # --- S12 remote (needs AG2) ---
            for t in range(1, NCORES):
                s12_block(t, f"t{t}")
            # --- S22 remote; RS_a + l1 partial under it ---
            for m in range(1, 5):
                rt = load_block(ag2_out, m, f"rs22_{m}")
                sym_group(n2_f8, rt, m, D2p, 2, f"s22_{m}")
                if m == 1:
                    # S11+S12 colsums all flushed -> reduce them under S22
                    nc.gpsimd.collective_compute(
                        "ReduceScatter", ALU.add, replica_groups=rg,
                        ins=[rs_in_a[:].opt()], outs=[rs_out_a[:].opt()],
                    )
                    # D1 side of the loss, precomputed while S22 runs
                    p2 = small.tile([P, NT], F32, tag="p2")
                    nc.sync.dma_start(p2[:], pt(p_dram[:]))
                    pm = small.tile([P, NT], F32, tag="pm")
                    nc.vector.tensor_scalar(pm[:], p2[:], -2.0, None, ALU.mult)
                    r1s = small.tile([P, NT], F32, tag="r1s")
                    nc.vector.reduce_sum(r1s[:], D1p[:], axis=mybir.AxisListType.X)
                    c1 = small.tile([P, NT], F32, tag="c1")
                    c2b = small.tile([P, NT], F32, tag="c2b")
                    nc.sync.dma_start(c1[:], pt(rs_out_a[0]))
                    nc.sync.dma_start(c2b[:], pt(rs_out_a[1]))
                    d1 = small.tile([P, NT], F32, tag="d1")
                    nc.vector.tensor_tensor(d1[:], r1s[:], c1[:], ALU.add)
                    l1 = small.tile([P, NT], F32, tag="l1")
                    nc.scalar.activation(l1[:], d1[:], AF.Ln, bias=negE2[:])
                    loss_a = small.tile([P, NT], F32, tag="loss_a")
                    nc.vector.tensor_scalar(loss_a[:], l1[:], 0.5, None, ALU.mult)
                    nc.vector.tensor_tensor(loss_a[:], loss_a[:], pm[:], ALU.add)
            flush_pending()
            # d2 partial (S22 rowsums + S12-colsum term) in parallel with RS_b
            r2s = small.tile([P, NT], F32, tag="r2s")
            nc.vector.reduce_sum(r2s[:], D2p[:], axis=mybir.AxisListType.X)
            d2 = small.tile([P, NT], F32, tag="d2")
            nc.vector.tensor_tensor(d2[:], r2s[:], c2b[:], ALU.add)

            nc.gpsimd.collective_compute(
                "ReduceScatter", ALU.add, replica_groups=rg,
                ins=[rs_in_b[:].opt()], outs=[rs_out_b[:].opt()],
            )

            # ---------------- final loss (tail) ----------------
            c2a = small.tile([P, NT], F32, tag="c2a")
            nc.sync.dma_start(c2a[:], pt(rs_out_b[:]))
            nc.vector.tensor_tensor(d2[:], d2[:], c2a[:], ALU.add)
            l2 = small.tile([P, NT], F32, tag="l2")
            nc.scalar.activation(l2[:], d2[:], AF.Ln, bias=negE2[:])
            loss = small.tile([P, NT], F32, tag="loss")
            nc.vector.tensor_scalar(loss[:], l2[:], 0.5, None, ALU.mult)
            nc.vector.tensor_tensor(loss[:], loss[:], loss_a[:], ALU.add)
            nc.sync.dma_start(pt(out[:]), loss[:])

    nc.finalize()
    return nc


@lru_cache(maxsize=1)
def _built():
    return _build()


def _kimajor(a):
    """[D, X] -> [ki=128, ko=8, X] contiguous device layout."""
    d, x = a.shape
    return np.ascontiguousarray(a.reshape(KO, P, x).transpose(1, 0, 2))


def _prep_inputs(z1, z2, fc1_w, fc1_b, fc2_w, fc2_b):
    f8 = ml_dtypes.float8_e4m3
    w1 = np.asarray(fc1_w, np.float32)
    w2 = np.asarray(fc2_w, np.float32)
    w1t = _kimajor(np.ascontiguousarray(w1.T).astype(f8))
    w2t = _kimajor(np.ascontiguousarray(w2.T).astype(f8))
    b1 = np.asarray(fc1_b, np.float32)
    # device computes (elu+1) @ W2.T; correct with the fp8-rounded W2 row sums
    b2p = (
        np.asarray(fc2_b, np.float32) - w2.astype(f8).astype(np.float32).sum(axis=1)
    ).astype(np.float32)
    in_maps = []
    for c in range(NCORES):
        sl = slice(c * BLK, (c + 1) * BLK)
        in_maps.append(
            {
                "z1t": _kimajor(np.asarray(z1[sl], np.float32).T.astype(f8)),
                "z2t": _kimajor(np.asarray(z2[sl], np.float32).T.astype(f8)),
                "w1t": w1t,
                "w2t": w2t,
                "b1": b1,
                "b2p": b2p,
            }
        )
    return in_maps


def _install_ntff_shim():
    """Register the axon NTFF profile hook (antenv.axon_hooks is absent in
    this image; rebuild it from trn_agent_boot's ctypes recipe)."""
    import sys
    import types

    if "antenv.axon_hooks" in sys.modules:
        return True
    try:
        import antenv
        from trn_agent_boot.trn_boot import _ntff_profile_via_ctypes

        hook = _ntff_profile_via_ctypes("/opt/axon/libaxon_pjrt.so")
        if hook is None:
            return False
        m = types.ModuleType("antenv.axon_hooks")
        m._hook = hook
        m.get_axon_ntff_profile_hook = lambda: m._hook
        m.set_axon_ntff_profile_hook = lambda h: setattr(m, "_hook", h)
        sys.modules["antenv.axon_hooks"] = m
        antenv.axon_hooks = m
        # artifact upload needs egress; neuter it for local profiling
        import concourse.bass_utils as _bu

        _bu.upload_artifacts = lambda tmpdir: f"file://{tmpdir}"
        return True
    except Exception as e:
        print(f"ntff shim unavailable: {e!r}")
        return False


def _run(in_maps, trace=False):
    nc = _built()
    if trace and not _install_ntff_shim():
        trace = False
    last = None
    for attempt in range(3):
        try:
            res = run_bass_kernel_spmd(nc, in_maps, list(range(NCORES)), trace=trace)
            if all(np.isfinite(res.results[c]["out"]).all() for c in range(NCORES)):
                return res
            print("nonfinite output, retrying")
        except Exception as e:  # device occasionally wedged from a prior process
            last = e
            if "UNRECOVERABLE" not in str(e) and "UNAVAILABLE" not in str(e):
                raise
            print(f"device error (attempt {attempt}): retrying")
    if last is not None:
        raise last
    return res


def kernel(z1, z2, fc1_w, fc1_b, fc2_w, fc2_b):
    in_maps = _prep_inputs(z1, z2, fc1_w, fc1_b, fc2_w, fc2_b)
    res = _run(in_maps, trace=os.environ.get("KERNEL_TRACE", "") == "1")
    if res.exec_time_ns is not None:
        print(f"HW exec time: {res.exec_time_ns} ns")
    out = np.concatenate([res.results[c]["out"] for c in range(NCORES)])
    return out.astype(np.float32)


# revision 27
# speedup vs baseline: 1.1205x; 1.0453x over previous
"""Trainium2 Bass kernel for nn_CLLayer (SimCLR-style contrastive loss).

Math (reference, tau=0.5):
    h1 = elu(z1 @ W1.T + b1) @ W2.T + b2 ; h2 likewise
    n1, n2 = row-normalized h1, h2
    l1_i = log(sum_j exp(2*n1_i.n1_j) + sum_j exp(2*n1_i.n2_j) - e^2) - 2*n1_i.n2_i
    l2_i = log(sum_j exp(2*n2_i.n2_j) + sum_j exp(2*n1_j.n2_i) - e^2) - 2*n1_i.n2_i
    out = 0.5*(l1+l2)

Strategy (row-parallel over N=8192, 1024 rows/core, 8 cores):
 - FP8(e4m3) DoubleRow matmuls everywhere (projection + similarity).
 - S11/S22 symmetric: cyclic per-core distribution at 512-row strip
   granularity using partition_id-driven dynamic DMA offsets. Core c's
   row strip r (r = 2c+h) computes cells (r, (r+e) mod 16) for e=0..8:
   e=0 is the local diagonal (rhs straight from SBUF, no AllGather
   dependency), e=1..7 full weight, e=8 at half weight (exp bias ln1/2;
   the partner strip computes the transposed cell also at half). Row
   sums are local (activation accum_out); the transposed halves arrive
   as per-strip column sums via ReduceScatter.
 - S12 not symmetric -> 8 full 1024-blocks, rotated (c+t)%8 so t=0 is
   the local block (SBUF rhs, no AllGather); its column sums feed l2.
 - exp tiles are fp8; [128,1024]-wide exp instructions wherever the
   bias is uniform across the two 512-col streams. Column sums are
   ones-vector DoubleRow matmuls accumulated in PSUM, deferred one
   group so the PE never waits on Act.
 - All DRAM<->SBUF layouts are [128, KO, x] (contiguous); host
   pre-arranges inputs. Normalized embeddings are scaled x16, cast fp8
   before the fp8 AllGather; sim psums are descaled in the exp
   (scale=2/256). pos diag p_i = n1_i.n2_i from a separate bf16 path.
 - Schedule: proj1(z1), proj1(z2), proj2(h1), norm(h1), AG1,
   proj2(h2), norm(h2), AG2, p-path + all-local sim cells (covers the
   AG1 mesh), S11 remote, S12 remote, RS(S11+S12 colsums) under S22
   remote, RS(S22 colsums), tail.

Host-side prep: K-major [ki, ko, x] arrangement, fp8 casts, and
b2' = b2 - sum_k W2_f8 so ELU is computed as relu(x) + min(exp(x),1)
(device ELU' = elu + 1).
"""

import math
import os
from functools import lru_cache

import ml_dtypes
import numpy as np

import concourse.bacc as bacc
import concourse.bass as bass
import concourse.mybir as mybir
import concourse.tile as tile
from concourse.bass_utils import run_bass_kernel_spmd

N, D = 8192, 1024
NCORES = 8
BLK = N // NCORES  # 1024
P = 128
KO = D // P  # 8 k-tiles
KO2 = KO // 2  # 4 double-row k-pairs
NT = BLK // P  # 8 i-tiles per core
E2 = float(np.exp(2.0))  # exp(1/tau), tau=0.5
SC = 2.0 / 256.0  # exp scale: tau and the 16x16 fp8 prescale
LN_HALF = float(math.log(0.5))
BF = mybir.dt.bfloat16
F32 = mybir.dt.float32
F8 = mybir.dt.float8e4
AF = mybir.ActivationFunctionType
ALU = mybir.AluOpType
DR = mybir.MatmulPerfMode.DoubleRow

# D1p slot layout: S11 uses 0..5 (h0: 5, h1: 6), S12 uses 6..13
S12_SLOT0 = 6
ND1 = 14
ND2 = 6


def _build():
    nc = bacc.Bacc("TRN2", target_bir_lowering=False, debug=False, num_devices=NCORES)

    # all tensors arrive pre-arranged as [ki=128, ko, x] (contiguous loads)
    z1t = nc.dram_tensor("z1t", [P, KO, BLK], F8, kind="ExternalInput")
    z2t = nc.dram_tensor("z2t", [P, KO, BLK], F8, kind="ExternalInput")
    w1t = nc.dram_tensor("w1t", [P, KO, D], F8, kind="ExternalInput")
    w2t = nc.dram_tensor("w2t", [P, KO, D], F8, kind="ExternalInput")
    b1 = nc.dram_tensor("b1", [D], F32, kind="ExternalInput")
    b2p = nc.dram_tensor("b2p", [D], F32, kind="ExternalInput")
    out = nc.dram_tensor("out", [BLK], F32, kind="ExternalOutput")

    pt = lambda ap: ap.rearrange("(t p) -> p t", p=P)  # [1024] -> [128, 8]

    with tile.TileContext(nc) as tc:
        with (
            tc.tile_pool(name="consts", bufs=1) as consts,
            tc.tile_pool(name="mats", bufs=1) as mats,
            tc.tile_pool(name="strip", bufs=1) as strip,
            tc.tile_pool(name="scratch", bufs=2) as scratch,
            tc.tile_pool(name="rhs", bufs=4) as rhsp,
            tc.tile_pool(name="expp", bufs=6) as expp,
            tc.tile_pool(name="small", bufs=1) as small,
            tc.tile_pool(name="psA", bufs=3, space="PSUM") as psA,
            tc.tile_pool(name="psB", bufs=2, space="PSUM") as psB,
            tc.tile_pool(name="dram", bufs=1, space="DRAM") as dram,
        ):
            pid_sp = nc.sync.partition_id()
            pid_gp = nc.gpsimd.partition_id()

            # ---------------- constants ----------------
            w1_sb = consts.tile([P, KO, D], F8)
            w2_sb = consts.tile([P, KO, D], F8)
            b1_sb = consts.tile([P, KO], F32)
            b2_sb = consts.tile([P, KO], F32)
            z1_sb = mats.tile([P, KO, BLK], F8, tag="z1")
            z2_sb = mats.tile([P, KO, BLK], F8, tag="z2")
            # two half-chunks: first matmuls start after chunk A lands
            nc.sync.dma_start(w1_sb[:, 0:4, :], w1t[:, 0:4, :])
            nc.sync.dma_start(z1_sb[:, 0:4, :], z1t[:, 0:4, :])
            nc.sync.dma_start(w1_sb[:, 4:8, :], w1t[:, 4:8, :])
            nc.sync.dma_start(z1_sb[:, 4:8, :], z1t[:, 4:8, :])
            nc.sync.dma_start(w2_sb[:], w2t[:])
            nc.sync.dma_start(z2_sb[:], z2t[:])
            nc.sync.dma_start(b1_sb[:], pt(b1[:]))
            nc.sync.dma_start(b2_sb[:], pt(b2p[:]))
            ones8 = consts.tile([P, 2, 16], F8)
            ones_bf = consts.tile([P, 1], BF)
            lnhalf = consts.tile([P, 1], F32)
            negE2 = consts.tile([P, 1], F32)
            nc.vector.memset(ones8[:], 1.0)
            nc.vector.memset(ones_bf[:], 1.0)
            nc.vector.memset(lnhalf[:], LN_HALF)
            nc.vector.memset(negE2[:], -E2)

            h1_sb = mats.tile([P, KO, BLK], BF, tag="h1")
            h2_sb = mats.tile([P, KO, BLK], BF, tag="h2")
            n1_f8 = mats.tile([P, KO, BLK], F8, tag="n1")
            n2_f8 = mats.tile([P, KO, BLK], F8, tag="n2")

            ag1_in = dram.tile([P, KO, BLK], F8)
            ag2_in = dram.tile([P, KO, BLK], F8)
            ag1_out = dram.tile([NCORES, P, KO, BLK], F8, addr_space="Shared")
            ag2_out = dram.tile([NCORES, P, KO, BLK], F8, addr_space="Shared")
            rs_in_a = dram.tile([NCORES, 2, BLK], F32)  # S11 / S12 colsums
            rs_in_b = dram.tile([NCORES, BLK], F32)  # S22 colsums
            rs_out_a = dram.tile([2, BLK], F32)
            rs_out_b = dram.tile([BLK], F32)
            p_dram = dram.tile([BLK], F32)

            # rowsum partials: one column per exp-instruction group
            D1p = strip.tile([P, NT, ND1], F32)
            D2p = strip.tile([P, NT, ND2], F32)
            nc.vector.memset(D1p[:], 0.0)
            nc.vector.memset(D2p[:], 0.0)

            # zero rs buffers: each core only exports colsums for the strips
            # it computed; the ReduceScatter sums whole buffers
            zt = consts.tile([1, BLK], F32)
            nc.vector.memset(zt[:], 0.0)
            for j in range(NCORES):
                nc.gpsimd.dma_start(rs_in_a[j, 0], zt[:])
                nc.gpsimd.dma_start(rs_in_a[j, 1], zt[:])
                nc.gpsimd.dma_start(rs_in_b[j], zt[:])

            rn_f = [
                small.tile([1, BLK], F32, tag=f"rn_f{i}", name=f"rn_f{i}")
                for i in range(2)
            ]

            def dr_multi(ps_list, lhs, tt, rhs_list):
                """K=1024 fp8 DoubleRow accumulation over several (ps, rhs)
                streams sharing the same stationary lhs tile per k-pair."""
                for k2 in range(KO2):
                    lslice = lhs[:, 2 * k2 : 2 * k2 + 2, bass.ts(tt, P)]
                    for ps_sl, (rt, col) in zip(ps_list, rhs_list):
                        nc.tensor.matmul(
                            ps_sl,
                            lslice,
                            rt[:, 2 * k2 : 2 * k2 + 2, bass.ds(col, 512)],
                            start=(k2 == 0),
                            stop=(k2 == KO2 - 1),
                            perf_mode=DR,
                        )

            # ------------ projection + normalize, per tensor ------------
            def proj_layer(w_sb, src, emit_ot, ots=None):
                for ot in ots if ots is not None else range(KO):
                    ps = psA.tile([P, 1024], F32, tag="ps_big")
                    dr_multi(
                        [ps[:, 0:512], ps[:, 512:1024]],
                        w_sb, ot, [(src, 0), (src, 512)],
                    )
                    emit_ot(ot, ps)

            def proj_layer_rounds(w_sb, src, emit_ot):
                """k2-outer over 3-ot rounds: consumes src k-chunks at load
                pace, so the first layer starts before its DMA completes."""
                for rnd in ((0, 1, 2), (3, 4, 5), (6, 7)):
                    pss = [
                        psA.tile([P, 1024], F32, tag="ps_big", name=f"psr{ot}")
                        for ot in rnd
                    ]
                    for k2 in range(KO2):
                        for i, ot in enumerate(rnd):
                            lslice = w_sb[:, 2 * k2 : 2 * k2 + 2, bass.ts(ot, P)]
                            for col in (0, 512):
                                nc.tensor.matmul(
                                    pss[i][:, col : col + 512],
                                    lslice,
                                    src[:, 2 * k2 : 2 * k2 + 2, bass.ds(col, 512)],
                                    start=(k2 == 0),
                                    stop=(k2 == KO2 - 1),
                                    perf_mode=DR,
                                )
                    for i, ot in enumerate(rnd):
                        emit_ot(ot, pss[i])

            def l1_emit(elu_sb):
                def emit(ot, ps):
                    bcol = b1_sb[:, ot : ot + 1]
                    e_t = scratch.tile([P, 1024], BF, tag="e_t", bufs=3)
                    r_t = scratch.tile([P, 1024], BF, tag="r_t", bufs=3)
                    nc.scalar.activation(e_t[:], ps[:], AF.Exp, bias=bcol)
                    nc.scalar.activation(r_t[:], ps[:], AF.Relu, bias=bcol)
                    nc.vector.tensor_scalar(e_t[:], e_t[:], 1.0, None, ALU.min)
                    nc.vector.tensor_tensor(elu_sb[:, ot, :], e_t[:], r_t[:], ALU.add)
                return emit

            def l2_emit(h_sb):
                def emit(ot, ps):
                    if ot % 2 == 0:
                        nc.scalar.activation(
                            h_sb[:, ot, :], ps[:], AF.Identity, bias=b2_sb[:, ot : ot + 1]
                        )
                    else:
                        nc.vector.tensor_scalar(
                            h_sb[:, ot, :], ps[:], b2_sb[:, ot : ot + 1], None, ALU.add
                        )
                return emit

            def normalize_pre(h_sb, rn_slot):
                """sumsq over d (bf16 ones-matmul on h*h) + DVE fast
                reciprocal -> y = 1/ssq. No Act-engine work."""
                ssps = [
                    psB.tile([16, 512], F32, name=f"ssps{rn_slot}_{c}", tag="ps_wide")
                    for c in range(2)
                ]
                for kt in range(KO):
                    sq = scratch.tile([P, BLK], BF, tag="sq")
                    nc.vector.tensor_tensor(sq[:], h_sb[:, kt, :], h_sb[:, kt, :], ALU.mult)
                    for ch in range(2):
                        nc.tensor.matmul(
                            ssps[ch][0:1, :],
                            ones_bf[:],
                            sq[:, bass.ts(ch, 512)],
                            start=(kt == 0),
                            stop=(kt == KO - 1),
                        )
                ssq_c = small.tile([1, BLK], F32, tag="ssq_c", name=f"ssq{rn_slot}")
                y_c = small.tile([1, BLK], F32, tag="y_c", name=f"y{rn_slot}")
                for ch in range(2):
                    nc.vector.tensor_copy(ssq_c[:, bass.ts(ch, 512)], ssps[ch][0:1, :])
                nc.vector.reciprocal_approx_fast(y_c[:], ssq_c[:])
                return y_c

            def normalize_post(h_sb, n_f8, rn_slot, y_c):
                """rn = sqrt(1/ssq) via Act Sqrt (scale=256 folds the x16 fp8
                prescale), broadcast, scale + fp8 cast."""
                rn16_bf = small.tile([1, BLK], BF, tag="rn16_bf", name=f"rn16_{rn_slot}")
                nc.scalar.activation(rn16_bf[:], y_c[:], AF.Sqrt, scale=256.0)
                nc.scalar.activation(rn_f[rn_slot][:], y_c[:], AF.Sqrt)
                rn_bc = scratch.tile([P, BLK], BF, tag="rnbc", bufs=2, name=f"rnbc{rn_slot}")
                nc.gpsimd.partition_broadcast(rn_bc[:], rn16_bf[:])
                for kt in range(KO):
                    nc.vector.tensor_tensor(
                        n_f8[:, kt, :], h_sb[:, kt, :], rn_bc[:], ALU.mult
                    )

            rg = [list(range(NCORES))]
            elu1 = mats.tile([P, KO, BLK], F8, tag="elu", name="elu1")
            elu2 = mats.tile([P, KO, BLK], F8, tag="z1", name="elu2")  # z1 dead post-l1

            # z1's full chain first so AG1 launches as early as possible; the
            # h1 Sqrt/broadcast interleaves between z2's projection halves so
            # it lands in its natural Act-queue slot without head-blocking
            l1e2 = l1_emit(elu2)
            proj_layer(w1_sb, z1_sb, l1_emit(elu1))
            proj_layer(w2_sb, elu1, l2_emit(h1_sb))
            y1 = normalize_pre(h1_sb, 0)
            proj_layer(w1_sb, z2_sb, l1e2, ots=range(0, 4))
            normalize_post(h1_sb, n1_f8, 0, y1)
            nc.sync.dma_start(ag1_in[:], n1_f8[:])
            nc.gpsimd.collective_compute(
                "AllGather", ALU.bypass, replica_groups=rg,
                ins=[ag1_in[:].opt()], outs=[ag1_out[:].opt()],
            )
            proj_layer(w1_sb, z2_sb, l1e2, ots=range(4, 8))
            proj_layer(w2_sb, elu2, l2_emit(h2_sb))
            y2 = normalize_pre(h2_sb, 1)
            normalize_post(h2_sb, n2_f8, 1, y2)
            nc.sync.dma_start(ag2_in[:], n2_f8[:])
            nc.gpsimd.collective_compute(
                "AllGather", ALU.bypass, replica_groups=rg,
                ins=[ag2_in[:].opt()], outs=[ag2_out[:].opt()],
            )

            # ------ p_i = n1_i . n2_i via bf16 h1*h2 and f32 1/norms ------
            pps = [
                psB.tile([16, 512], F32, name=f"pps{c}", tag="ps_wide") for c in range(2)
            ]
            for kt in range(KO):
                hq = scratch.tile([P, BLK], BF, tag="sq", name=f"hq{kt}")
                nc.vector.tensor_tensor(hq[:], h1_sb[:, kt, :], h2_sb[:, kt, :], ALU.mult)
                for ch in range(2):
                    nc.tensor.matmul(
                        pps[ch][0:1, :],
                        ones_bf[:],
                        hq[:, bass.ts(ch, 512)],
                        start=(kt == 0),
                        stop=(kt == KO - 1),
                    )
            for ch in range(2):
                sl = bass.ts(ch, 512)
                p_c = small.tile([1, 512], F32, tag="p_c", bufs=2, name=f"p_c{ch}")
                nc.vector.tensor_copy(p_c[:], pps[ch][0:1, :])
                nc.vector.tensor_tensor(p_c[:], p_c[:], rn_f[0][:, sl], ALU.mult)
                nc.vector.tensor_tensor(p_c[:], p_c[:], rn_f[1][:, sl], ALU.mult)
                nc.gpsimd.dma_start(p_dram[ch * 512 : (ch + 1) * 512], p_c[:])

            # ---------------- similarity passes ----------------
            # colsums deferred to the next group so the PE never waits on
            # the Act engine's exp outputs
            pending = []

            def flush_pending():
                while pending:
                    pending.pop(0)()

            def colsum(jdyn, hh, rs_sel, src_aps, nm):
                """PSUM-accumulated fp8 ones DoubleRow colsum of [128,2,512]
                exp slices -> dynamic rs slot (block jdyn, 512-col half hh).
                rs_sel: 0/1 -> rs_in_a slot, 2 -> rs_in_b."""

                def emit():
                    cps = psB.tile([16, 512], F32, tag="ps_wide", name=f"cps{nm}")
                    for i, ap in enumerate(src_aps):
                        nc.tensor.matmul(
                            cps[:], ones8[:], ap,
                            start=(i == 0), stop=(i == len(src_aps) - 1),
                            perf_mode=DR,
                        )
                    cst = scratch.tile([1, 512], F32, tag="cst", bufs=2, name=f"cst{nm}")
                    nc.vector.tensor_copy(cst[:], cps[0:1, :])
                    if rs_sel == 2:
                        dst = rs_in_b[bass.ds(jdyn, 1), hh * 512 : hh * 512 + 512]
                    else:
                        dst = rs_in_a[
                            bass.ds(jdyn, 1), rs_sel : rs_sel + 1,
                            hh * 512 : hh * 512 + 512,
                        ]
                    nc.gpsimd.dma_start(dst, cst[:])

                pending.append(emit)

            def load_block(ag, m, nm):
                jj = (pid_sp + m) % NCORES
                t = rhsp.tile([P, KO, BLK], F8, tag="rhs", name=nm)
                nc.sync.dma_start(t[:, 0:4, :], ag[bass.ds(jj, 1), :, 0:4, :])
                nc.sync.dma_start(t[:, 4:8, :], ag[bass.ds(jj, 1), :, 4:8, :])
                return t

            def sym_group(n_f8, rt, m, Dp, rs_sel, nm):
                """One cyclic m-group of a symmetric matrix: cells of both
                local row halves h against block (c+m): strips 2c+2m, 2c+2m+1.

                m=0: local rhs (rt = n_f8): h0 pair (e0,e1), h1 lone diag e0.
                m=1..3: h0 pair (e2m,e2m+1), h1 pair (e2m-1,e2m), full wt.
                m=4: h0 lone e8 (half wt), h1 pair (e7, e8 at half wt).
                Dp accum slots: h0 -> m; h1 -> m (m<4), m4 -> 4,5."""
                first = True
                ex_half = {0: [], 1: []}  # strip half -> colsum source APs
                for h in (0, 1):
                    # (col, half-weight?, colsummed?) per stream
                    if m == 0:
                        streams = (
                            [(0, False, False), (512, False, True)]
                            if h == 0
                            else [(512, False, False)]
                        )
                    elif m < 4:
                        streams = [(0, False, True), (512, False, True)]
                    elif h == 0:
                        streams = [(0, True, True)]
                    else:
                        streams = [(0, False, True), (512, True, True)]
                    nst = len(streams)
                    wide = nst == 2 and streams[0][1] == streams[1][1]
                    if wide:
                        ext = [
                            expp.tile([P, 4, 1024], F8, tag="exw", bufs=4,
                                      name=f"xw{nm}{h}")
                        ]
                    else:
                        ext = [
                            expp.tile([P, 4, 512], F8, tag="exn", bufs=4,
                                      name=f"xn{nm}{h}{i}")
                            for i in range(nst)
                        ]
                    for tl in range(4):
                        tt = 4 * h + tl
                        ps = psA.tile([P, 1024], F32, tag="ps_big", name=f"p{nm}{h}{tl}")
                        dr_multi(
                            [ps[:, 512 * i : 512 * (i + 1)] for i in range(nst)],
                            n_f8, tt, [(rt, col) for col, _, _ in streams],
                        )
                        if first:
                            flush_pending()
                            first = False
                        if wide:
                            nc.scalar.activation(
                                ext[0][:, tl, :], ps[:], AF.Exp, scale=SC,
                                bias=(lnhalf[:] if streams[0][1] else 0.0),
                                accum_out=Dp[:, tt, m : m + 1],
                            )
                        else:
                            for i, (col, halfw, _) in enumerate(streams):
                                sl = (4 + i) if (m == 4 and h == 1) else m
                                nc.scalar.activation(
                                    ext[i][:, tl, :],
                                    ps[:, 512 * i : 512 * (i + 1)],
                                    AF.Exp, scale=SC,
                                    bias=(lnhalf[:] if halfw else 0.0),
                                    accum_out=Dp[:, tt, sl : sl + 1],
                                )
                    # colsum sources per strip half of block (c+m):
                    # stream i covers strip half i, except m0 (h0 stream1 ->
                    # half 1) and m4-h0 lone (e8 -> half 0).
                    for i, (col, halfw, summed) in enumerate(streams):
                        if not summed:
                            continue
                        hh = i
                        if m == 0:
                            hh = 1
                        elif m == 4 and h == 0:
                            hh = 0
                        for tp in range(2):
                            if wide:
                                ex_half[hh].append(
                                    ext[0][:, 2 * tp : 2 * tp + 2,
                                           512 * i : 512 * i + 512]
                                )
                            else:
                                ex_half[hh].append(ext[i][:, 2 * tp : 2 * tp + 2, :])
                jgp = (pid_gp + m) % NCORES
                for hh in (0, 1):
                    if ex_half[hh]:
                        colsum(jgp, hh, rs_sel, ex_half[hh], f"{nm}h{hh}")

            def s12_block(t, nm):
                if t == 0:
                    rt = n2_f8
                else:
                    rt = load_block(ag2_out, t, f"r{nm}")
                exf = [
                    expp.tile([P, 2, 1024], F8, tag="exf", bufs=6, name=f"xf{nm}_{i}")
                    for i in range(4)
                ]
                for tt in range(NT):
                    ps = psA.tile([P, 1024], F32, tag="ps_big", name=f"pf{nm}_{tt}")
                    dr_multi(
                        [ps[:, 0:512], ps[:, 512:1024]],
                        n1_f8, tt, [(rt, 0), (rt, 512)],
                    )
                    if tt == 0:
                        flush_pending()
                    nc.scalar.activation(
                        exf[tt // 2][:, tt % 2, :], ps[:], AF.Exp, scale=SC,
                        accum_out=D1p[:, tt, S12_SLOT0 + t : S12_SLOT0 + t + 1],
                    )
                jgp = (pid_gp + t) % NCORES
                for hh in range(2):
                    colsum(
                        jgp, hh, 1,
                        [e[:, :, hh * 512 : hh * 512 + 512] for e in exf],
                        f"f{nm}_{hh}",
                    )

            # --- all-local sim work first: covers the AG1/AG2 meshes ---
            sym_group(n1_f8, n1_f8, 0, D1p, 0, "s11_0")
            s12_block(0, "t0")
            sym_group(n2_f8, n2_f8, 0, D2p, 2, "s22_0")
            # --- S11 remote (needs AG1) ---
            for m in range(1, 5):
                rt = load_block(ag1_out, m, f"rs11_{m}")
                sym_group(n1_f8, rt, m, D1p, 0, f"s11_{m}")
            # --- S12 remote (needs AG2) ---
            for t in range(1, NCORES):
                s12_block(t, f"t{t}")
            # --- S22 remote; RS_a + l1 partial under it ---
            for m in range(1, 5):
                rt = load_block(ag2_out, m, f"rs22_{m}")
                sym_group(n2_f8, rt, m, D2p, 2, f"s22_{m}")
                if m == 1:
                    # S11+S12 colsums all flushed -> reduce them under S22
                    nc.gpsimd.collective_compute(
                        "ReduceScatter", ALU.add, replica_groups=rg,
                        ins=[rs_in_a[:].opt()], outs=[rs_out_a[:].opt()],
                    )
                    # D1 side of the loss, precomputed while S22 runs
                    p2 = small.tile([P, NT], F32, tag="p2")
                    nc.sync.dma_start(p2[:], pt(p_dram[:]))
                    pm = small.tile([P, NT], F32, tag="pm")
                    nc.vector.tensor_scalar(pm[:], p2[:], -2.0, None, ALU.mult)
                    r1s = small.tile([P, NT], F32, tag="r1s")
                    nc.vector.reduce_sum(r1s[:], D1p[:], axis=mybir.AxisListType.X)
                    c1 = small.tile([P, NT], F32, tag="c1")
                    c2b = small.tile([P, NT], F32, tag="c2b")
                    nc.sync.dma_start(c1[:], pt(rs_out_a[0]))
                    nc.sync.dma_start(c2b[:], pt(rs_out_a[1]))
                    d1 = small.tile([P, NT], F32, tag="d1")
                    nc.vector.tensor_tensor(d1[:], r1s[:], c1[:], ALU.add)
                    l1 = small.tile([P, NT], F32, tag="l1")
                    nc.scalar.activation(l1[:], d1[:], AF.Ln, bias=negE2[:])
                    loss_a = small.tile([P, NT], F32, tag="loss_a")
                    nc.vector.tensor_scalar(loss_a[:], l1[:], 0.5, None, ALU.mult)
                    nc.vector.tensor_tensor(loss_a[:], loss_a[:], pm[:], ALU.add)
            flush_pending()
            # d2 partial (S22 rowsums + S12-colsum term) in parallel with RS_b
            r2s = small.tile([P, NT], F32, tag="r2s")
            nc.vector.reduce_sum(r2s[:], D2p[:], axis=mybir.AxisListType.X)
            d2 = small.tile([P, NT], F32, tag="d2")
            nc.vector.tensor_tensor(d2[:], r2s[:], c2b[:], ALU.add)

            nc.gpsimd.collective_compute(
                "ReduceScatter", ALU.add, replica_groups=rg,
                ins=[rs_in_b[:].opt()], outs=[rs_out_b[:].opt()],
            )

            # ---------------- final loss (tail) ----------------
            c2a = small.tile([P, NT], F32, tag="c2a")
            nc.sync.dma_start(c2a[:], pt(rs_out_b[:]))
            nc.vector.tensor_tensor(d2[:], d2[:], c2a[:], ALU.add)
            l2 = small.tile([P, NT], F32, tag="l2")
            nc.scalar.activation(l2[:], d2[:], AF.Ln, bias=negE2[:])
            loss = small.tile([P, NT], F32, tag="loss")
            nc.vector.tensor_scalar(loss[:], l2[:], 0.5, None, ALU.mult)
            nc.vector.tensor_tensor(loss[:], loss[:], loss_a[:], ALU.add)
            nc.sync.dma_start(pt(out[:]), loss[:])

    nc.finalize()
    return nc


@lru_cache(maxsize=1)
def _built():
    return _build()


def _kimajor(a):
    """[D, X] -> [ki=128, ko=8, X] contiguous device layout."""
    d, x = a.shape
    return np.ascontiguousarray(a.reshape(KO, P, x).transpose(1, 0, 2))


def _prep_inputs(z1, z2, fc1_w, fc1_b, fc2_w, fc2_b):
    f8 = ml_dtypes.float8_e4m3
    w1 = np.asarray(fc1_w, np.float32)
    w2 = np.asarray(fc2_w, np.float32)
    w1t = _kimajor(np.ascontiguousarray(w1.T).astype(f8))
    w2t = _kimajor(np.ascontiguousarray(w2.T).astype(f8))
    b1 = np.asarray(fc1_b, np.float32)
    # device computes (elu+1) @ W2.T; correct with the fp8-rounded W2 row sums
    b2p = (
        np.asarray(fc2_b, np.float32) - w2.astype(f8).astype(np.float32).sum(axis=1)
    ).astype(np.float32)
    in_maps = []
    for c in range(NCORES):
        sl = slice(c * BLK, (c + 1) * BLK)
        in_maps.append(
            {
                "z1t": _kimajor(np.asarray(z1[sl], np.float32).T.astype(f8)),
                "z2t": _kimajor(np.asarray(z2[sl], np.float32).T.astype(f8)),
                "w1t": w1t,
                "w2t": w2t,
                "b1": b1,
                "b2p": b2p,
            }
        )
    return in_maps


def _install_ntff_shim():
    """Register the axon NTFF profile hook (antenv.axon_hooks is absent in
    this image; rebuild it from trn_agent_boot's ctypes recipe)."""
    import sys
    import types

    if "antenv.axon_hooks" in sys.modules:
        return True
    try:
        import antenv
        from trn_agent_boot.trn_boot import _ntff_profile_via_ctypes

        hook = _ntff_profile_via_ctypes("/opt/axon/libaxon_pjrt.so")
        if hook is None:
            return False
        m = types.ModuleType("antenv.axon_hooks")
        m._hook = hook
        m.get_axon_ntff_profile_hook = lambda: m._hook
        m.set_axon_ntff_profile_hook = lambda h: setattr(m, "_hook", h)
        sys.modules["antenv.axon_hooks"] = m
        antenv.axon_hooks = m
        # artifact upload needs egress; neuter it for local profiling
        import concourse.bass_utils as _bu

        _bu.upload_artifacts = lambda tmpdir: f"file://{tmpdir}"
        return True
    except Exception as e:
        print(f"ntff shim unavailable: {e!r}")
        return False


def _run(in_maps, trace=False):
    nc = _built()
    if trace and not _install_ntff_shim():
        trace = False
    last = None
    for attempt in range(3):
        try:
            res = run_bass_kernel_spmd(nc, in_maps, list(range(NCORES)), trace=trace)
            if all(np.isfinite(res.results[c]["out"]).all() for c in range(NCORES)):
                return res
            print("nonfinite output, retrying")
        except Exception as e:  # device occasionally wedged from a prior process
            last = e
            if "UNRECOVERABLE" not in str(e) and "UNAVAILABLE" not in str(e):
                raise
            print(f"device error (attempt {attempt}): retrying")
    if last is not None:
        raise last
    return res


def kernel(z1, z2, fc1_w, fc1_b, fc2_w, fc2_b):
    in_maps = _prep_inputs(z1, z2, fc1_w, fc1_b, fc2_w, fc2_b)
    res = _run(in_maps, trace=os.environ.get("KERNEL_TRACE", "") == "1")
    if res.exec_time_ns is not None:
        print(f"HW exec time: {res.exec_time_ns} ns")
    out = np.concatenate([res.results[c]["out"] for c in range(NCORES)])
    return out.astype(np.float32)
